# revision 1
# baseline (speedup 1.0000x reference)
"""Trainium2 Bass kernel for the BaselinePreprocessor problem.

Computes, for full inputs:
  fused = concat([interp(vision->T), interp(proprio->T), imu], -1)  # [64,1024,550]
  vox_mean = mean(occupancy grid 64^3 of 10k points)               # scalar
  out = concat([fused, vox_mean bcast], -1)                        # [64,1024,551]

Strategy: pure data parallel over batch (8 cores x 8 batches). Linear
interpolation along time is a sparse linear map -> dense TensorE matmuls with
host-precomputed weight matrices (constants derived from shapes only). The
voxel histogram is built per-core via one indirect-DMA scatter of ones into a
DRAM grid, then reduced on-device.
"""

import numpy as np

import concourse.bacc as bacc
import concourse.bass as bass
import concourse.mybir as mybir
import concourse.tile as tile
from concourse.bass_utils import run_bass_kernel_spmd

F32 = mybir.dt.float32
F16 = mybir.dt.float16
BF16 = mybir.dt.bfloat16
I32 = mybir.dt.int32
ALU = mybir.AluOpType

N_CORES = 8
B_PER_CORE = 8
T = 1024
LV, CV = 64, 512     # vision input time-len, channels
LP, CP = 256, 32     # proprio
CI = 6               # imu channels (identity interp: L == T)
C_OUT = 551
GRID = 64
NVOX = GRID * GRID * GRID  # 262144
NPTS = 10000
NPTS_CORE = NPTS // N_CORES          # 1250 points scattered per core
PTS_P, PTS_F = 125, NPTS_CORE // 125  # [125, 10] per-core point layout
N_TILES = T // 128         # 8 time tiles of 128 rows


def _interp_weights_T(L: int) -> np.ndarray:
    """W^T [L, T] with W the [T, L] linear-interp matrix (align_corners)."""
    scale = np.float32((L - 1) / (T - 1))
    pos = np.arange(T, dtype=np.float32) * scale
    lo = np.clip(np.floor(pos).astype(np.int32), 0, L - 1)
    hi = np.minimum(lo + 1, L - 1)
    w = (pos - lo.astype(np.float32)).astype(np.float32)
    wt = np.zeros((L, T), dtype=np.float32)
    np.add.at(wt, (lo, np.arange(T)), np.float32(1.0) - w)
    np.add.at(wt, (hi, np.arange(T)), w)
    return np.ascontiguousarray(wt)


def _proprio_chunks_needed(j: int) -> list[int]:
    """Which K=128 row chunks of W_p^T have nonzeros for time tile j."""
    lo0 = (128 * j * (LP - 1)) // (T - 1)
    lo1 = (128 * j + 127) * (LP - 1) // (T - 1)
    hi1 = min(lo1 + 1, LP - 1)
    ks = []
    if lo0 < 128:
        ks.append(0)
    if hi1 >= 128:
        ks.append(1)
    return ks


def _emit(nc: bass.Bass, tc: tile.TileContext, ctx, debug_vox: bool = False):
    vision = nc.declare_dram_parameter("vision", [B_PER_CORE, LV, CV], F32, isOutput=False)
    proprio = nc.declare_dram_parameter("proprio", [B_PER_CORE, LP, CP], F32, isOutput=False)
    imu = nc.declare_dram_parameter("imu", [B_PER_CORE, T, CI], F32, isOutput=False)
    points = nc.declare_dram_parameter("points", [NPTS_CORE, 3], F32, isOutput=False)
    # vision interp weights in an fp16 hi/lo pair: W = wvh + wvl to ~2^-24
    # relative, so three fp16 matmuls (hi@hi + hi@lo + lo@hi) reproduce the
    # fp32 product to ~1e-6 absolute at far lower PE cost than fp32 matmul.
    wvh = nc.declare_dram_parameter("wvh", [LV, T], F16, isOutput=False)
    wvl = nc.declare_dram_parameter("wvl", [LV, T], F16, isOutput=False)
    wp = nc.declare_dram_parameter("wp", [LP, T], F32, isOutput=False)
    out = nc.declare_dram_parameter("out", [B_PER_CORE, T, C_OUT], F32, isOutput=True)

    # bf16 occupancy grid (0/1 values are exact; halves the AllReduce bytes)
    grid = nc.dram_tensor("grid", [NVOX, 1], BF16)
    grid_2d = grid[:].rearrange("(p f) o -> p (f o)", p=128)  # [128, 2048]
    grid_sh = nc.dram_tensor("grid_sh", [NVOX, 1], BF16, addr_space="Shared")
    grid_sh_2d = grid_sh[:].rearrange("(p f) o -> p (f o)", p=128)

    const = ctx.enter_context(tc.tile_pool(name="const", bufs=1))
    work = ctx.enter_context(tc.tile_pool(name="work", bufs=1))
    stream = ctx.enter_context(tc.tile_pool(name="stream", bufs=3))
    outp = ctx.enter_context(tc.tile_pool(name="outp", bufs=6))
    psumv = ctx.enter_context(tc.tile_pool(name="psumv", bufs=3, space="PSUM"))
    psump = ctx.enter_context(tc.tile_pool(name="psump", bufs=2, space="PSUM"))
    psums = ctx.enter_context(tc.tile_pool(name="psums", bufs=1, space="PSUM"))

    # ---------------- voxel occupancy scalar ----------------
    # zero the DRAM grid
    zer = const.tile([128, 2048], BF16)
    nc.vector.memset(zer[:], 0.0)
    nc.scalar.dma_start(out=grid_2d, in_=zer[:])

    # load points as [125, 80, 3]
    pts = work.tile([PTS_P, PTS_F, 3], F32)
    nc.scalar.dma_start(out=pts[:], in_=points[:].rearrange("(p f) c -> p f c", p=PTS_P))

    # per-coordinate voxel index, exactly replicating the reference arithmetic:
    # q = clip(trunc((p + 2) * 16), 0, 63); computed as clip-then-floor which
    # is equivalent (trunc==floor for the surviving non-negative range).
    # floor(x) for x in [0, 63]: round-trip through int32 (rounding mode of
    # the cast may be trunc or nearest) then subtract 1 wherever the result
    # exceeds x — exact either way.
    q = []
    ji = work.tile([PTS_P, PTS_F], I32)
    gt = work.tile([PTS_P, PTS_F], F32)
    for c in range(3):
        qc = work.tile([PTS_P, PTS_F], F32, tag=f"q{c}")
        nc.vector.tensor_scalar(qc[:], pts[:, :, c], 2.0, 16.0, ALU.add, ALU.mult)
        nc.vector.tensor_scalar(qc[:], qc[:], 63.0, 0.0, ALU.min, ALU.max)
        rt = work.tile([PTS_P, PTS_F], F32, tag=f"rt{c}")
        nc.vector.tensor_copy(out=ji[:], in_=qc[:])
        nc.vector.tensor_copy(out=rt[:], in_=ji[:])
        nc.vector.tensor_tensor(gt[:], rt[:], qc[:], ALU.is_gt)
        nc.vector.tensor_tensor(qc[:], rt[:], gt[:], ALU.subtract)
        q.append(qc)
    acc = work.tile([PTS_P, PTS_F], F32)
    nc.vector.tensor_scalar(acc[:], q[0][:], 64.0, None, ALU.mult)
    nc.vector.tensor_tensor(acc[:], acc[:], q[1][:], ALU.add)
    nc.vector.tensor_scalar(acc[:], acc[:], 64.0, None, ALU.mult)
    nc.vector.tensor_tensor(acc[:], acc[:], q[2][:], ALU.add)
    idx = work.tile([PTS_P, PTS_F], I32)
    nc.vector.tensor_copy(out=idx[:], in_=acc[:])  # exact integers -> exact

    # Scatter ones: the HW indirect DMA consumes ONE offset per partition
    # (writing the source's free dim contiguously there), so each call
    # scatters up to 128 points — one call per index column. Each core only
    # scatters its own 1/8 of the points; AllReduce(max) below unions the
    # partial occupancy grids.
    ones_pts = const.tile([PTS_P, 1], BF16)
    nc.vector.memset(ones_pts[:], 1.0)
    for f in range(PTS_F):
        nc.gpsimd.indirect_dma_start(
            out=grid[:],
            out_offset=bass.IndirectOffsetOnAxis(ap=idx[:, f:f + 1], axis=0),
            in_=ones_pts[:],
            in_offset=None,
        )
    nc.gpsimd.collective_compute(
        "AllReduce",
        ALU.max,
        replica_groups=[list(range(N_CORES))],
        ins=[grid[:]],
        outs=[grid_sh[:]],
    )

    if debug_vox:
        dbg_idx = nc.declare_dram_parameter("dbg_idx", [PTS_P, PTS_F], I32, isOutput=True)
        nc.sync.dma_start(out=dbg_idx[:], in_=idx[:])
        dbg_q = nc.declare_dram_parameter("dbg_q", [3, PTS_P, PTS_F], F32, isOutput=True)
        for c in range(3):
            nc.sync.dma_start(out=dbg_q[c], in_=q[c][:])

    # read back and reduce to the mean scalar, broadcast to [128,1]
    rb = work.tile([128, 2048], BF16)
    nc.scalar.dma_start(out=rb[:], in_=grid_sh_2d)

    if debug_vox:
        dbg_grid = nc.declare_dram_parameter("dbg_grid", [128, 2048], F32, isOutput=True)
        nc.sync.dma_start(out=dbg_grid[:], in_=rb[:])
    red = work.tile([128, 1], F32)
    nc.vector.tensor_reduce(red[:], rb[:], axis=mybir.AxisListType.X, op=ALU.add)
    ones_col = const.tile([128, 1], F32)
    nc.vector.memset(ones_col[:], 1.0)
    ps = psums.tile([1, 1], F32, tag="ps_scalar")
    nc.tensor.matmul(out=ps[:], lhsT=red[:], rhs=ones_col[:], start=True, stop=True)
    s_sb = work.tile([1, 1], F32)
    nc.vector.tensor_copy(out=s_sb[:], in_=ps[:])
    scale_row = const.tile([1, 128], F32)
    nc.vector.memset(scale_row[:], 1.0 / NVOX)  # 2**-18, exact
    pb = psums.tile([128, 1], F32, tag="ps_bcast")
    nc.tensor.matmul(out=pb[:], lhsT=scale_row[:], rhs=s_sb[:], start=True, stop=True)
    vox = work.tile([128, 1], F32)
    nc.vector.tensor_copy(out=vox[:], in_=pb[:])
    # The summary column is written by its own tiny per-batch DMAs so the
    # main output stream never waits on the voxel-scalar chain.
    vox_row = work.tile([128, N_TILES], F32)
    nc.vector.tensor_copy(out=vox_row[:], in_=vox[:].to_broadcast([128, N_TILES]))
    for b in range(B_PER_CORE):
        nc.sync.dma_start(
            out=out[b, :, 550:551].rearrange("(j p) o -> p (j o)", p=128),
            in_=vox_row[:],
        )

    # ---------------- interpolation via matmul ----------------
    wvh_sb = const.tile([LV, T], F16)
    nc.scalar.dma_start(out=wvh_sb[:], in_=wvh[:])
    wvl_sb = const.tile([LV, T], F16)
    nc.scalar.dma_start(out=wvl_sb[:], in_=wvl[:])
    wp0_sb = const.tile([128, T], F32)
    nc.scalar.dma_start(out=wp0_sb[:], in_=wp[0:128, :])
    wp1_sb = const.tile([128, T], F32)
    nc.scalar.dma_start(out=wp1_sb[:], in_=wp[128:256, :])
    wp_sb = [wp0_sb, wp1_sb]

    # all batches' proprio, laid out [k-row 128, chunk 2, batch 8, chan 32]:
    # one cross-batch matmul (N = 8*32) per (time tile, nonzero chunk).
    pall = const.tile([128, 2, B_PER_CORE, CP], F32)
    for k in range(2):
        nc.scalar.dma_start(
            out=pall[:, k, :, :],
            in_=proprio[:, 128 * k:128 * (k + 1), :].rearrange("b p c -> p b c"),
        )
    pp_tiles = []
    for j in range(N_TILES):
        js = slice(j * 128, (j + 1) * 128)
        ppj = psump.tile([128, B_PER_CORE, CP], F32, tag="pp")
        ks = _proprio_chunks_needed(j)
        for i, k in enumerate(ks):
            nc.tensor.matmul(
                out=ppj[:],
                lhsT=wp_sb[k][:, js],
                rhs=pall[:, k, :, :],
                start=(i == 0),
                stop=(i == len(ks) - 1),
            )
        pp_sb = work.tile([128, B_PER_CORE, CP], F32, tag=f"ppsb{j}", name=f"ppsb{j}")
        nc.vector.tensor_copy(out=pp_sb[:], in_=ppj[:])
        pp_tiles.append(pp_sb)

    for b in range(B_PER_CORE):
        vb = stream.tile([LV, CV], F32, tag="vb")
        nc.scalar.dma_start(out=vb[:], in_=vision[b])
        vh = stream.tile([LV, CV], F16, tag="vh")
        nc.vector.tensor_copy(out=vh[:], in_=vb[:])
        vtmp = stream.tile([LV, CV], F32, tag="vtmp")
        nc.vector.tensor_copy(out=vtmp[:], in_=vh[:])
        nc.vector.tensor_tensor(vtmp[:], vb[:], vtmp[:], ALU.subtract)
        vl = stream.tile([LV, CV], F16, tag="vl")
        nc.vector.tensor_copy(out=vl[:], in_=vtmp[:])
        ib = stream.tile([128, N_TILES, CI], F32, tag="ib")
        nc.scalar.dma_start(out=ib[:], in_=imu[b].rearrange("(j p) c -> p j c", j=N_TILES))

        for j in range(N_TILES):
            js = slice(j * 128, (j + 1) * 128)
            pv = psumv.tile([128, CV], F32, tag="pv")
            nc.tensor.matmul(out=pv[:], lhsT=wvh_sb[:, js], rhs=vh[:], start=True, stop=False)
            nc.tensor.matmul(out=pv[:], lhsT=wvh_sb[:, js], rhs=vl[:], start=False, stop=False)
            nc.tensor.matmul(out=pv[:], lhsT=wvl_sb[:, js], rhs=vh[:], start=False, stop=True)

            ob = outp.tile([128, 550], F32, tag="ob")
            nc.vector.tensor_copy(out=ob[:, 0:CV], in_=pv[:])
            nc.vector.tensor_copy(out=ob[:, CV:CV + CP], in_=pp_tiles[j][:, b, :])
            nc.vector.tensor_copy(out=ob[:, 544:550], in_=ib[:, j, :])
            nc.sync.dma_start(out=out[b, js, 0:550], in_=ob[:])


_CACHE: dict[str, object] = {}


def _get_nc() -> bass.Bass:
    if "nc" not in _CACHE:
        from contextlib import ExitStack

        # Bacc (not plain Bass): its finalize() legalizes sync waits (HW
        # allows at most one wait per instruction; extras are split into
        # event-semaphore instructions).
        nc = bacc.Bacc(None, num_devices=N_CORES)
        with ExitStack() as ctx:
            tc = ctx.enter_context(tile.TileContext(nc))
            _emit(nc, tc, ctx)
        if not nc.is_finalized():
            nc.finalize()
        _CACHE["nc"] = nc
    return _CACHE["nc"]  # type: ignore[return-value]


def _run(inputs: dict, trace: bool = False):
    vision = np.ascontiguousarray(np.asarray(inputs["vision"], dtype=np.float32))
    proprio = np.ascontiguousarray(np.asarray(inputs["proprio"], dtype=np.float32))
    imu = np.ascontiguousarray(np.asarray(inputs["imu"], dtype=np.float32))
    points = np.ascontiguousarray(np.asarray(inputs["points"], dtype=np.float32))
    wv = _interp_weights_T(LV)
    wvh = wv.astype(np.float16)
    wvl = (wv - wvh.astype(np.float32)).astype(np.float16)
    wp = _interp_weights_T(LP)

    nc = _get_nc()
    in_maps = []
    for i in range(N_CORES):
        sl = slice(i * B_PER_CORE, (i + 1) * B_PER_CORE)
        psl = slice(i * NPTS_CORE, (i + 1) * NPTS_CORE)
        in_maps.append({
            "vision": vision[sl],
            "proprio": proprio[sl],
            "imu": imu[sl],
            "points": np.ascontiguousarray(points[psl]),
            "wvh": wvh,
            "wvl": wvl,
            "wp": wp,
        })
    res = run_bass_kernel_spmd(nc, in_maps, list(range(N_CORES)), trace=trace)
    full = np.concatenate([res.results[i]["out"] for i in range(N_CORES)], axis=0)
    return full, res


def kernel(**inputs) -> np.ndarray:
    full, _ = _run(inputs)
    return full



# revision 6
# speedup vs baseline: 1.0198x; 1.0198x over previous
"""Trainium2 Bass kernel for the BaselinePreprocessor problem.

Computes, for full inputs:
  fused = concat([interp(vision->T), interp(proprio->T), imu], -1)  # [64,1024,550]
  vox_mean = mean(occupancy grid 64^3 of 10k points)               # scalar
  out = concat([fused, vox_mean bcast], -1)                        # [64,1024,551]

Strategy (v2):
- Pure data parallel over batch (8 cores x 8 batches).
- Interp along time is a dense fp16 TensorE matmul with host-precomputed
  weights (one matmul per time tile, tolerance 2e-2 >> fp16 error ~1e-3).
- Inputs are host-transposed/cast so every DMA has large contiguous
  descriptors (imu -> [T,B,6] gives 192B rows instead of 24B).
- Output written as ONE DMA per time tile covering all 8 batches
  ([128, 8, 550] -> 2200B descriptors), alternating sync/scalar queues.
- Voxel occupancy without DRAM scatter: points are replicated to all
  cores; core i counts distinct voxels in slab [i*32768,(i+1)*32768) via
  one-hot is_equal tiles + 79 accumulating matmuls into one PSUM tile
  (count[hi,lo] += [hi(idx_k)==hi]*[lo(idx_k)==lo]), then Sign+accum_out
  clamps and row-sums, and a tiny AllReduce(add) sums slab counts.
  The vox column is patched in with per-tile column DMAs.
"""

import numpy as np

import concourse.bacc as bacc
import concourse.bass as bass
import concourse.mybir as mybir
import concourse.tile as tile
from concourse.bass_utils import run_bass_kernel_spmd

F32 = mybir.dt.float32
F16 = mybir.dt.float16
BF16 = mybir.dt.bfloat16
I32 = mybir.dt.int32
ALU = mybir.AluOpType
ACT = mybir.ActivationFunctionType

N_CORES = 8
B = 8                      # batches per core
T = 1024
LV, CV = 64, 512           # vision input time-len, channels
LP, CP = 256, 32           # proprio
CI = 6                     # imu channels (identity interp: L == T)
C_OUT = 551
GRID = 64
NVOX = GRID * GRID * GRID  # 262144
NPTS = 10000
PTS_F = 79                 # points laid out [128, 79] (padded to 10112)
NPTS_PAD = 128 * PTS_F
SLAB = NVOX // N_CORES     # 32768 voxels per core's slab
HI, LO = 128, 256          # slab voxel index split: idx_local = hi*256 + lo
N_TILES = T // 128         # 8 time tiles of 128 rows
OH_CHUNKS = [20, 20, 20, 19]  # point-column chunking for one-hot builds


def _interp_weights_T(L: int) -> np.ndarray:
    """W^T [L, T] with W the [T, L] linear-interp matrix (align_corners)."""
    scale = np.float32((L - 1) / (T - 1))
    pos = np.arange(T, dtype=np.float32) * scale
    lo = np.clip(np.floor(pos).astype(np.int32), 0, L - 1)
    hi = np.minimum(lo + 1, L - 1)
    w = (pos - lo.astype(np.float32)).astype(np.float32)
    wt = np.zeros((L, T), dtype=np.float32)
    np.add.at(wt, (lo, np.arange(T)), np.float32(1.0) - w)
    np.add.at(wt, (hi, np.arange(T)), w)
    return np.ascontiguousarray(wt)


def _proprio_chunks_needed(j: int) -> list[int]:
    """Which K=128 row chunks of W_p^T have nonzeros for time tile j."""
    lo0 = (128 * j * (LP - 1)) // (T - 1)
    lo1 = (128 * j + 127) * (LP - 1) // (T - 1)
    hi1 = min(lo1 + 1, LP - 1)
    ks = []
    if lo0 < 128:
        ks.append(0)
    if hi1 >= 128:
        ks.append(1)
    return ks


def _emit(nc: bass.Bass, tc: tile.TileContext, ctx):
    vis = nc.declare_dram_parameter("vis", [LV, B, CV], F16, isOutput=False)
    pro = nc.declare_dram_parameter("pro", [LP, B, CP], F16, isOutput=False)
    imu = nc.declare_dram_parameter("imu", [T, B, CI], F32, isOutput=False)
    pts = nc.declare_dram_parameter("pts", [NPTS_PAD, 3], F32, isOutput=False)
    wv = nc.declare_dram_parameter("wv", [LV, T], F16, isOutput=False)
    wp = nc.declare_dram_parameter("wp", [LP, T], F16, isOutput=False)
    slab = nc.declare_dram_parameter("slab", [128, 1], I32, isOutput=False)
    out = nc.declare_dram_parameter("out", [B, T, C_OUT], F32, isOutput=True)

    cnt_dram = nc.dram_tensor("cnt", [1, 128], F32)
    cnt_sh = nc.dram_tensor("cnt_sh", [1, 128], F32, addr_space="Shared")

    const = ctx.enter_context(tc.tile_pool(name="const", bufs=1))
    vxw = ctx.enter_context(tc.tile_pool(name="vxw", bufs=1))
    ohp = ctx.enter_context(tc.tile_pool(name="ohp", bufs=2))
    rp = ctx.enter_context(tc.tile_pool(name="rp", bufs=2))
    outp = ctx.enter_context(tc.tile_pool(name="outp", bufs=3))
    psumv = ctx.enter_context(tc.tile_pool(name="psumv", bufs=3, space="PSUM"))
    psump = ctx.enter_context(tc.tile_pool(name="psump", bufs=2, space="PSUM"))
    psumg = ctx.enter_context(tc.tile_pool(name="psumg", bufs=1, space="PSUM"))
    psums = ctx.enter_context(tc.tile_pool(name="psums", bufs=1, space="PSUM"))

    # ---------------- constant loads (scalar=ACT HWDGE queue) ----------------
    wv_sb = const.tile([LV, T], F16)
    nc.scalar.dma_start(out=wv_sb[:], in_=wv[:])
    wp_sb = const.tile([128, 2, T], F16)
    nc.scalar.dma_start(out=wp_sb[:], in_=wp[:].rearrange("(k p) t -> p k t", p=128))
    vh_sb = const.tile([LV, B, CV], F16)
    nc.scalar.dma_start(out=vh_sb[:], in_=vis[:])
    pro_sb = const.tile([128, 2, B, CP], F16)
    nc.scalar.dma_start(out=pro_sb[:], in_=pro[:].rearrange("(k p) b c -> p k b c", p=128))
    imu_sb = const.tile([128, N_TILES, B, CI], F32)
    nc.scalar.dma_start(out=imu_sb[:], in_=imu[:].rearrange("(j p) b c -> p j b c", p=128))
    pts_sb = vxw.tile([128, PTS_F, 3], F32)
    nc.sync.dma_start(out=pts_sb[:], in_=pts[:].rearrange("(p f) c -> p f c", p=128))
    slab_sb = vxw.tile([128, 1, 1], I32)
    nc.sync.dma_start(out=slab_sb[:, :, 0], in_=slab[:])

    # iotas for the one-hot builds
    iota_hi = const.tile([128, 1, HI], I32)
    nc.gpsimd.iota(iota_hi[:], pattern=[[1, HI]], base=0, channel_multiplier=0)
    iota_lo = const.tile([128, 1, LO], I32)
    nc.gpsimd.iota(iota_lo[:], pattern=[[1, LO]], base=0, channel_multiplier=0)
    ones_col = const.tile([128, 1], F32)
    nc.gpsimd.memset(ones_col[:], 1.0)
    ones_row = const.tile([1, 128], F32)
    nc.gpsimd.memset(ones_row[:], 1.0)

    # ---------------- voxel index math (vector) ----------------
    # q_c = clip(trunc((p_c + 2) * 16), 0, 63), computed clip-then-floor
    # (equivalent: trunc==floor on the surviving non-negative range).
    # floor via int32 round-trip (any rounding mode) + is_gt correction.
    q = []
    ji = vxw.tile([128, PTS_F, 1], I32)
    gt = vxw.tile([128, PTS_F, 1], F32)
    for c in range(3):
        qc = vxw.tile([128, PTS_F, 1], F32, tag=f"q{c}")
        nc.vector.tensor_scalar(qc[:, :, 0], pts_sb[:, :, c], 2.0, 16.0, ALU.add, ALU.mult)
        nc.vector.tensor_scalar(qc[:], qc[:], 63.0, 0.0, ALU.min, ALU.max)
        rt = vxw.tile([128, PTS_F, 1], F32, tag=f"rt{c}")
        nc.vector.tensor_copy(out=ji[:], in_=qc[:])
        nc.vector.tensor_copy(out=rt[:], in_=ji[:])
        nc.vector.tensor_tensor(gt[:], rt[:], qc[:], ALU.is_gt)
        nc.vector.tensor_tensor(qc[:], rt[:], gt[:], ALU.subtract)
        q.append(qc)
    acc = vxw.tile([128, PTS_F, 1], F32)
    nc.vector.tensor_scalar(acc[:], q[0][:], 64.0, None, ALU.mult)
    nc.vector.tensor_tensor(acc[:], acc[:], q[1][:], ALU.add)
    nc.vector.tensor_scalar(acc[:], acc[:], 64.0, None, ALU.mult)
    nc.vector.tensor_tensor(acc[:], acc[:], q[2][:], ALU.add)
    idx_i = vxw.tile([128, PTS_F, 1], I32)
    nc.vector.tensor_copy(out=idx_i[:], in_=acc[:])  # exact integers -> exact
    # slab-local index; out-of-slab points self-mask (hi outside [0,128))
    nc.vector.tensor_tensor(
        idx_i[:], idx_i[:], slab_sb[:].to_broadcast([128, PTS_F, 1]), ALU.subtract
    )
    hi_i = vxw.tile([128, PTS_F, 1], I32)
    nc.vector.tensor_scalar(hi_i[:], idx_i[:], 8, None, ALU.arith_shift_right)
    lo_i = vxw.tile([128, PTS_F, 1], I32)
    nc.vector.tensor_scalar(lo_i[:], idx_i[:], 255, None, ALU.bitwise_and)

    # ---------------- proprio prepass: pp[j] for all batches ----------------
    pp_tiles = []
    for j in range(N_TILES):
        js = slice(j * 128, (j + 1) * 128)
        ppj = psump.tile([128, B, CP], F32, tag="pp")
        ks = _proprio_chunks_needed(j)
        for i, k in enumerate(ks):
            nc.tensor.matmul(
                out=ppj[:],
                lhsT=wp_sb[:, k, js],
                rhs=pro_sb[:, k, :, :],
                start=(i == 0),
                stop=(i == len(ks) - 1),
            )
        pp_sb = const.tile([128, B, CP], F32, tag=f"ppsb{j}", name=f"ppsb{j}")
        nc.vector.tensor_copy(out=pp_sb[:], in_=ppj[:])
        pp_tiles.append(pp_sb)

    # ---------------- main stream: one output tile per time tile ----------------
    grid_ps = psumg.tile([128, LO], F32)
    n_chunks = len(OH_CHUNKS)
    chunk_start = [sum(OH_CHUNKS[:c]) for c in range(n_chunks)]
    oh_tiles = {}
    r_tiles = {}

    def emit_onehot_chunk(c):
        f0, fn = chunk_start[c], OH_CHUNKS[c]
        oh = ohp.tile([128, max(OH_CHUNKS), HI], BF16, tag="oh")
        nc.vector.tensor_tensor(
            oh[:, 0:fn, :],
            hi_i[:, f0:f0 + fn, :].to_broadcast([128, fn, HI]),
            iota_hi[:].to_broadcast([128, fn, HI]),
            ALU.is_equal,
        )
        r = rp.tile([128, max(OH_CHUNKS), LO], BF16, tag="r")
        nc.vector.tensor_tensor(
            r[:, 0:fn, :],
            lo_i[:, f0:f0 + fn, :].to_broadcast([128, fn, LO]),
            iota_lo[:].to_broadcast([128, fn, LO]),
            ALU.is_equal,
        )
        oh_tiles[c] = oh
        r_tiles[c] = r

    def emit_grid_mms(c):
        f0, fn = chunk_start[c], OH_CHUNKS[c]
        for f in range(fn):
            nc.tensor.matmul(
                out=grid_ps[:],
                lhsT=oh_tiles[c][:, f, :],
                rhs=r_tiles[c][:, f, :],
                start=(f0 + f == 0),
                stop=(f0 + f == NPTS_PAD // 128 - 1),
            )

    for j in range(N_TILES):
        js = slice(j * 128, (j + 1) * 128)
        ob = outp.tile([128, B, C_OUT], F32, tag="ob")
        for b in range(B):
            pv = psumv.tile([128, CV], F32, tag="pv")
            nc.tensor.matmul(
                out=pv[:], lhsT=wv_sb[:, js], rhs=vh_sb[:, b, :], start=True, stop=True
            )
            nc.vector.tensor_copy(out=ob[:, b, 0:CV], in_=pv[:])
        nc.vector.tensor_copy(out=ob[:, :, CV:CV + CP], in_=pp_tiles[j][:])
        nc.vector.tensor_copy(out=ob[:, :, 544:550], in_=imu_sb[:, j, :, :])
        eng = nc.sync if j % 2 == 0 else nc.scalar
        eng.dma_start(
            out=out[:, js, 0:550].rearrange("b p c -> p b c"), in_=ob[:, :, 0:550]
        )
        # interleave voxel one-hot builds + grid matmuls behind the stream
        if j < n_chunks:
            emit_onehot_chunk(j)
            emit_grid_mms(j)

    # ---------------- voxel count -> AllReduce -> vox column patches ----------------
    occ = vxw.tile([128, LO], BF16)
    red = vxw.tile([128, 1], F32)
    # Sign(count): counts >= 0 -> exactly the 0/1 occupancy; accum_out row-sums it
    nc.scalar.activation(out=occ[:], in_=grid_ps[:], func=ACT.Sign, accum_out=red[:])
    cnt_ps = psums.tile([1, 1], F32, tag="cnt")
    nc.tensor.matmul(out=cnt_ps[:], lhsT=red[:], rhs=ones_col[:], start=True, stop=True)
    cnt_sb = vxw.tile([1, 128], F32)
    nc.gpsimd.memset(cnt_sb[:], 0.0)
    nc.scalar.activation(out=cnt_sb[:, 0:1], in_=cnt_ps[:], func=ACT.Copy)
    nc.gpsimd.dma_start(out=cnt_dram[:], in_=cnt_sb[:])
    nc.gpsimd.collective_compute(
        "AllReduce",
        ALU.add,
        replica_groups=[list(range(N_CORES))],
        ins=[cnt_dram[:]],
        outs=[cnt_sh[:]],
    )
    cnt_rb = vxw.tile([1, 128], F32)
    nc.gpsimd.dma_start(out=cnt_rb[:], in_=cnt_sh[:])
    vox1 = vxw.tile([1, 1], F32)
    nc.gpsimd.tensor_scalar(vox1[:], cnt_rb[:, 0:1], 1.0 / NVOX, None, ALU.mult)
    vox_pb = psums.tile([128, 1], F32, tag="voxb")
    nc.tensor.matmul(out=vox_pb[:], lhsT=ones_row[:], rhs=vox1[:], start=True, stop=True)
    vox_row = vxw.tile([128, B], F32)
    nc.scalar.activation(
        out=vox_row[:], in_=vox_pb[:].to_broadcast([128, B]), func=ACT.Copy
    )
    for j in range(N_TILES):
        js = slice(j * 128, (j + 1) * 128)
        eng = nc.sync if j % 2 == 0 else nc.scalar
        eng.dma_start(
            out=out[:, js, 550:551].rearrange("b p o -> p (b o)"), in_=vox_row[:]
        )


_CACHE: dict[str, object] = {}


def _get_nc() -> bass.Bass:
    if "nc" not in _CACHE:
        from contextlib import ExitStack

        # Bacc (not plain Bass): its finalize() legalizes sync waits (HW
        # allows at most one wait per instruction; extras are split into
        # event-semaphore instructions).
        nc = bacc.Bacc(None, num_devices=N_CORES)
        with ExitStack() as ctx:
            tc = ctx.enter_context(tile.TileContext(nc))
            _emit(nc, tc, ctx)
        if not nc.is_finalized():
            nc.finalize()
        _CACHE["nc"] = nc
    return _CACHE["nc"]  # type: ignore[return-value]


def _run(inputs: dict, trace: bool = False):
    vision = np.asarray(inputs["vision"], dtype=np.float32)
    proprio = np.asarray(inputs["proprio"], dtype=np.float32)
    imu = np.asarray(inputs["imu"], dtype=np.float32)
    points = np.asarray(inputs["points"], dtype=np.float32)[:NPTS]
    # pad the point list with copies of point 0: duplicates never change
    # the occupancy count
    pts_pad = np.concatenate(
        [points, np.broadcast_to(points[0], (NPTS_PAD - NPTS, 3))], axis=0
    )
    pts_pad = np.ascontiguousarray(pts_pad)
    wv16 = _interp_weights_T(LV).astype(np.float16)
    wp16 = _interp_weights_T(LP).astype(np.float16)

    nc = _get_nc()
    in_maps = []
    for i in range(N_CORES):
        sl = slice(i * B, (i + 1) * B)
        in_maps.append({
            "vis": np.ascontiguousarray(
                vision[sl].transpose(1, 0, 2).astype(np.float16)),
            "pro": np.ascontiguousarray(
                proprio[sl].transpose(1, 0, 2).astype(np.float16)),
            "imu": np.ascontiguousarray(imu[sl].transpose(1, 0, 2)),
            "pts": pts_pad,
            "wv": wv16,
            "wp": wp16,
            "slab": np.full((128, 1), i * SLAB, dtype=np.int32),
        })
    res = run_bass_kernel_spmd(nc, in_maps, list(range(N_CORES)), trace=trace)
    full = np.concatenate([res.results[i]["out"] for i in range(N_CORES)], axis=0)
    return full, res


def kernel(**inputs) -> np.ndarray:
    full, _ = _run(inputs)
    return full
